# revision 10
# baseline (speedup 1.0000x reference)
"""Masked-loss kernel for nn_MLoss_9715216024200 on 8 Trainium2 NeuronCores.

loss = sum(where(y[...,0]>0.5, (y-x)^2 - a*x^2, 0)) + a*sum(x[...,0]^2)
with x,y f32 (256, 10647, 5); output is a f32 scalar.

Sharding: flatten both tensors to cells (5 contiguous f32 each), pad with
256 zero-cells (neutral: y0=0 -> mask 0, x=0 -> no bg term), reshape to
(8 cores, 128 partitions, 2662 cells).  The host also interleaves the two
tensors per tile ([y_tile | x_tile] in one DRAM tensor) so each tile
needs ONE DMA instruction: a single SP issue queue then sustains the
stream (per-DMA issue cost ~700ns SEQ+HWDGE < per-tile transfer) and the
stream runs gapless at the 360 GB/s roofline.

Mask algebra: with m = (y0>0.5) per cell and mbar = 1-m,
  loss = sum(m*d^2) - a*sum(m*x_{1..4}^2) + a*sum(mbar*x0^2)
(the masked feature-0 x^2 term cancels against the all-cells background
term).  Per tile (cells per TILE_SIZES; a taper shrinks the end):
  DVE :  m5 = bf16(y0>0.5) broadcast to 5 features (2x mode)
         mb0 = bf16(y0<=0.5) (1-wide)
         d  = y - x (f32, bf16 out);  dm = d * m5 (2x) -> dm buffer
  Pool:  xm = x_{1..4}*m5_{1..4} (strided) -> xm buffer
         x0m = x0*mb0 -> x0m span buffer
Reductions ((m*v)^2 == m*v^2 for 0/1 m):
  per group:  acc_sq[g]  = sum(dm^2),  acc_sq2[g] = sum(xm^2)
  per span :  acc_sq3[s] = sum(x0m^2)   (spans cover several groups)
each on ACT (Square+accum) or DVE (stt accumulate) per config; late
groups lean on DVE so ACT's 557ns/op fixed costs stay off the
post-stream critical path.  Host combines in f64:
total = sum(sq) - a*sum(sq2) + a*sum(sq3).
"""
import sys

for _p in ('/opt/trn_rl_repo',):
    if _p in sys.path:
        sys.path.remove(_p)
    sys.path.insert(0, _p)

import numpy as np

B, C, F = 256, 10647, 5
THRESH = 0.5
ALPHA = 0.1
N_CORES = 8
P = 128
CELLS = B * C                      # 2,725,632
CELLS_PER_PART = 2662              # 8*128*2662 = 2,725,888
PAD_CELLS = N_CORES * P * CELLS_PER_PART - CELLS   # 256
FD = CELLS_PER_PART * F            # 13310 elems per partition per core


def _default_config():
    suffix = [96, 72, 56, 40, 28, 16, 8]
    main_total = CELLS_PER_PART - sum(suffix)
    n_main = 18
    base = main_total // n_main
    rem = main_total - base * n_main
    tiles = [base + (1 if i < rem else 0) for i in range(n_main)] + suffix
    return dict(
        tiles=tiles,
        # (n_tiles_in_group, sq_engine, sq2_engine) per group, in order
        groups=[(3, 'act', 'act')] * 6 +
               [(2, 'act', 'act'), (2, 'act', 'act'),
                (2, 'dve', 'dve'), (1, 'dve', 'dve')],
        # spans for the x0m reduction: tiles per span, engine
        x0_span=6,
        x0_eng_last='dve',  # engine for the final (partial) span
        defer_last=1,       # defer TTRs of the last k dve-groups
        xm_dve_tiles=1,     # last k tiles: xm on DVE (single-engine endgame)
        out_split=None,     # group index: out1 covers groups < split
        bufs=(12, 8, 4),
    )


CONFIG = _default_config()
_compiled = None


def _build(cfg=None):
    from contextlib import ExitStack
    import concourse.tile as tile
    from concourse import bacc, mybir

    cfg = cfg or CONFIG
    tiles = cfg['tiles']
    groups = cfg['groups']
    assert sum(tiles) == CELLS_PER_PART
    assert sum(g[0] for g in groups) == len(tiles)
    n_groups = len(groups)
    tile_group = [(gi, k) for gi, (gn, _, _) in enumerate(groups)
                  for k in range(gn)]
    xm_dve = set(range(len(tiles) - cfg['xm_dve_tiles'], len(tiles)))
    sqa = float(np.sqrt(ALPHA))

    nc = bacc.Bacc("TRN2", target_bir_lowering=False, debug=False,
                   enable_asserts=True, num_devices=N_CORES)
    xy_d = nc.dram_tensor("xy", [P, 2 * FD], mybir.dt.float32,
                          kind="ExternalInput").ap()
    # o columns: [sq(g), sq2(g)] pairs then sq3 spans
    _nspans = (len(tiles) + cfg['x0_span'] - 1) // cfg['x0_span']
    o_d = nc.dram_tensor("o", [P, 2 * n_groups + _nspans], mybir.dt.float32,
                         kind="ExternalOutput").ap()

    f32 = mybir.dt.float32
    bf16 = mybir.dt.bfloat16
    Sq = mybir.ActivationFunctionType.Square
    Alu = mybir.AluOpType

    # x0m spans: split tile indices into runs of x0_span tiles
    spans = []
    i = 0
    while i < len(tiles):
        spans.append(list(range(i, min(i + cfg['x0_span'], len(tiles)))))
        i += cfg['x0_span']
    n_spans = len(spans)
    tile_span = {}
    for si, ts_ in enumerate(spans):
        for k, tt in enumerate(ts_):
            tile_span[tt] = (si, k)
    NCOL = 2 * n_groups + n_spans

    def emit_sq(eng, buf, nel, col):
        scr = sp.tile([P, nel], bf16, tag="scr")
        if eng == 'act':
            nc.scalar.activation(scr[:], buf[:], Sq, accum_out=col)
        else:
            nc.vector.scalar_tensor_tensor(
                scr[:], buf[:], 1.0, buf[:], op0=Alu.mult, op1=Alu.mult,
                accum_out=col)

    with tile.TileContext(nc) as tc, ExitStack() as ctx:
        xyp = ctx.enter_context(tc.tile_pool(name="xy", bufs=cfg['bufs'][0]))
        wp = ctx.enter_context(tc.tile_pool(name="work", bufs=cfg['bufs'][1]))
        sp = ctx.enter_context(tc.tile_pool(name="scratch",
                                            bufs=cfg['bufs'][2]))
        ap_ = ctx.enter_context(tc.tile_pool(name="acc", bufs=1))

        acc = ap_.tile([P, NCOL], f32)

        deferred = []
        off = 0
        gdm = gxm = gx0 = None
        gdoff = gxoff = g0off = 0
        for t, cells in enumerate(tiles):
            fd = cells * F
            g, k_in_g = tile_group[t]
            gn = groups[g][0]
            gcells = sum(tiles[t - k_in_g:t - k_in_g + gn])
            si, k_in_s = tile_span[t]
            scells = sum(tiles[j] for j in spans[si])
            xyt = xyp.tile([P, 2 * fd], f32, tag="xyt")
            nc.sync.dma_start(xyt[:], xy_d[:, 2 * off:2 * off + 2 * fd])
            yt = xyt[:, :fd]
            xt = xyt[:, fd:]
            off += fd

            if k_in_g == 0:
                gdm = wp.tile([P, gcells * F], bf16, tag="dmg")
                gxm = wp.tile([P, gcells * 4], bf16, tag="xmg")
                gdoff = gxoff = 0
            if k_in_s == 0:
                gx0 = wp.tile([P, scells], bf16, tag="x0g")
                g0off = 0

            # DVE: 5-wide mask, 1-wide complement mask
            m5 = wp.tile([P, fd], bf16, tag="m5")
            y0s = yt[:, 0::F]
            y0b = y0s.unsqueeze(2).broadcast_to((P, cells, F))
            nc.vector.tensor_scalar(
                m5[:].rearrange("p (k f) -> p k f", f=F), y0b,
                THRESH, None, op0=Alu.is_gt)
            mb0 = wp.tile([P, cells], bf16, tag="mb0")
            nc.vector.tensor_scalar(mb0[:], y0s, THRESH, None, op0=Alu.is_le)

            # DVE: d = y - x (bf16 out), dm = d*m5 (2x)
            dt_ = wp.tile([P, fd], bf16, tag="d")
            nc.vector.tensor_tensor(dt_[:], yt, xt, op=Alu.subtract)
            nc.vector.tensor_tensor(gdm[:, gdoff:gdoff + fd], dt_[:], m5[:],
                                    op=Alu.mult)

            # Pool: xm over features 1..4 (strided views), x0m = x0*mb0
            x14 = xt.rearrange("p (k f) -> p k f", f=F)[:, :, 1:F]
            m14 = m5[:].rearrange("p (k f) -> p k f", f=F)[:, :, 1:F]
            xm_eng = nc.vector if t in xm_dve else nc.gpsimd
            xm_eng.tensor_tensor(
                gxm[:, gxoff:gxoff + cells * 4].rearrange(
                    "p (k f) -> p k f", f=F - 1),
                x14, m14, op=Alu.mult)
            x0m_eng = nc.vector if t in xm_dve else nc.gpsimd
            x0m_eng.tensor_tensor(gx0[:, g0off:g0off + cells], xt[:, 0::F],
                                  mb0[:], op=Alu.mult)
            gdoff += fd
            gxoff += cells * 4
            g0off += cells

            last_dve_pending = (
                sum(1 for gg in range(g, n_groups)
                    if 'dve' in (groups[gg][1], groups[gg][2]))
                <= cfg['defer_last'])
            if k_in_g == gn - 1:
                if last_dve_pending and (
                        'dve' in (groups[g][1], groups[g][2])):
                    deferred.append(
                        ('g', groups[g][1], gdm, gcells * 5, 2 * g,
                         groups[g][2], gxm, gcells * 4, 2 * g + 1))
                else:
                    emit_sq(groups[g][1], gdm, gcells * 5,
                            acc[:, 2 * g:2 * g + 1])
                    emit_sq(groups[g][2], gxm, gcells * 4,
                            acc[:, 2 * g + 1:2 * g + 2])
            if k_in_s == len(spans[si]) - 1:
                col = acc[:, 2 * n_groups + si:2 * n_groups + si + 1]
                eng = cfg['x0_eng_last'] if si == n_spans - 1 else 'act'
                if si == n_spans - 1 and eng == 'dve':
                    deferred.append(('s', eng, gx0, scells, None, None,
                                     None, None, None))
                else:
                    emit_sq(eng, gx0, scells, col)

        for item in deferred:
            kind, e1, b1, n1, c1, e2, b2, n2, c2 = item
            if kind == 'g':
                emit_sq(e1, b1, n1, acc[:, c1:c1 + 1])
                emit_sq(e2, b2, n2, acc[:, c2:c2 + 1])
            else:
                col = acc[:, NCOL - 1:NCOL]
                emit_sq(e1, b1, n1, col)

        nc.sync.dma_start(o_d[:], acc[:])

    nc.compile()
    nc._mloss_cfg = dict(n_groups=n_groups, n_spans=n_spans, tiles=tiles)
    return nc


def _shard_xy(x: np.ndarray, y: np.ndarray, tiles) -> list[np.ndarray]:
    """Per core: [P, 2*FD] with per-tile interleave [y_tile | x_tile]."""
    pad = np.zeros(PAD_CELLS * F, dtype=np.float32)
    xf = np.concatenate([x.reshape(-1), pad]).reshape(N_CORES, P, FD)
    yf = np.concatenate([y.reshape(-1), pad]).reshape(N_CORES, P, FD)
    out = np.empty((N_CORES, P, 2 * FD), dtype=np.float32)
    off = 0
    for cells in tiles:
        fd = cells * F
        out[:, :, 2 * off:2 * off + fd] = yf[:, :, off:off + fd]
        out[:, :, 2 * off + fd:2 * off + 2 * fd] = xf[:, :, off:off + fd]
        off += fd
    return [np.ascontiguousarray(out[i]) for i in range(N_CORES)]


def kernel(x: np.ndarray, y: np.ndarray) -> np.ndarray:
    global _compiled
    if _compiled is None:
        _compiled = _build()
    nc = _compiled

    from concourse.bass_utils import run_bass_kernel_spmd

    xys = _shard_xy(np.asarray(x, dtype=np.float32),
                    np.asarray(y, dtype=np.float32), nc._mloss_cfg['tiles'])
    in_maps = [{"xy": xys[i]} for i in range(N_CORES)]
    res = run_bass_kernel_spmd(nc, in_maps, core_ids=list(range(N_CORES)))

    ng = nc._mloss_cfg['n_groups']
    ns = nc._mloss_cfg['n_spans']
    total = np.float64(0.0)
    for r in res.results:
        o = r["o"].astype(np.float64).reshape(P, 2 * ng + ns)
        total += o[:, 0:2 * ng:2].sum()
        total -= ALPHA * o[:, 1:2 * ng:2].sum()
        total += ALPHA * o[:, 2 * ng:].sum()
    return np.float32(total)


# revision 13
# speedup vs baseline: 1.0562x; 1.0562x over previous
"""Masked-loss kernel for nn_MLoss_9715216024200 on 8 Trainium2 NeuronCores.

loss = sum(where(y[...,0]>0.5, (y-x)^2 - a*x^2, 0)) + a*sum(x[...,0]^2)
with x,y f32 (256, 10647, 5); output is a f32 scalar.

Sharding: flatten both tensors to cells (5 contiguous f32 each), pad with
256 zero-cells (mathematically neutral: y0=0 -> mask 0, x=0 -> no bg term),
reshape to (8 cores, 128 partitions, 2662 cells).  Each core streams its
13 MiB at the ~360 GB/s HBM roofline while three compute engines split the
elementwise work (every engine under the per-tile DMA time):

  per 127-cell tile (down to telescoped tail tiles):
    GpSimd: m5  = bf16(y0 > 0.5) replicated to all 5 features (contiguous)
            xs0 = bf16(sqrt(a)*x0)  -> tail slice of the group dmx buffer
            xm  = x * m5 for every 4th tile and the late tiles
    DVE:    d   = y - x   (f32 1x, bf16 out)
            dm  = d * m5  (bf16 2x) -> head slice of the group dmx buffer
            xm  = x * m5  (mixed 1x, bf16 out) for the remaining tiles
  per reduction GROUP of 1-3 consecutive tiles (ScalarE Square+accum_out,
  fp32 accumulate; grouping amortizes the ~370ns fixed cost per
  accumulate -- 187ns accumulator read + SBUF-access init):
            acc1[g] = sum(dmx^2) = sum((m*d)^2) + a*sum(x0^2)
            acc2[g] = sum(xm^2)  (unscaled; host applies a)
  the last two groups' acc2 run as DVE scalar_tensor_tensor accumulates,
  deferred past the loop, so the endgame after the final DMA is short.

m*v^2 == (m*v)^2 because m is 0/1, which is what lets the fused
Square-accumulate do all reductions.  Small tiles keep the DMA->accumulate
pipeline latency low (the last accumulate lands ~4.7us after the final
byte); grouped reductions keep ScalarE's fixed costs amortized.  bf16
intermediates cost ~2e-6 relative error on the final sum.  Host combines:
total = sum(acc1) - a*sum(acc2), in f64 over 8 cores x 128 partitions.
"""
import sys

for _p in ('/opt/trn_rl_repo',):
    if _p in sys.path:
        sys.path.remove(_p)
    sys.path.insert(0, _p)

import numpy as np

B, C, F = 256, 10647, 5
THRESH = 0.5
ALPHA = 0.1
N_CORES = 8
P = 128
CELLS = B * C                      # 2,725,632
CELLS_PER_PART = 2662              # ceil to 8*128*2662 = 2,725,888
PAD_CELLS = N_CORES * P * CELLS_PER_PART - CELLS   # 256
FD = CELLS_PER_PART * F            # 13310 elems per partition per core

TILE_SIZES = [121, 129] + [127] * 14 + [218, 162, 129, 125]
assert sum(TILE_SIZES) == CELLS_PER_PART
N_TILES = len(TILE_SIZES)
# reduction groups over consecutive tiles (one sq/sq2 pair per group)
GROUP_OF = [3, 3, 3, 3, 2, 2, 1, 1, 1, 1]
assert sum(GROUP_OF) == N_TILES
N_GROUPS = len(GROUP_OF)
_tile_group = [(gi, k) for gi, gn in enumerate(GROUP_OF) for k in range(gn)]
XM_ON_POOL = {3, 7, 11, 15, 17, 19}   # tiles whose xm runs on GpSimd
M5_ON_DVE = {0}       # tile 0's mask on DVE (Pool's broadcast would gate
                      # the pipeline head)
SQ2_ON_DVE = set()    # mid groups: sq2 as DVE stt (unused in final config)
TTR_TAIL = 2          # last k groups: sq2 as deferred DVE stt
REV_TTR = False
BUFS = [8, 8, 8, 4]

_compiled = None


def _build():
    from contextlib import ExitStack
    import concourse.tile as tile
    from concourse import bacc, mybir

    sqa = float(np.sqrt(ALPHA))

    nc = bacc.Bacc("TRN2", target_bir_lowering=False, debug=False,
                   enable_asserts=True, num_devices=N_CORES)
    x_d = nc.dram_tensor("x", [P, FD], mybir.dt.float32, kind="ExternalInput").ap()
    y_d = nc.dram_tensor("y", [P, FD], mybir.dt.float32, kind="ExternalInput").ap()
    o_d = nc.dram_tensor("o", [P, 2 * N_GROUPS], mybir.dt.float32,
                         kind="ExternalOutput").ap()

    f32 = mybir.dt.float32
    bf16 = mybir.dt.bfloat16
    Sq = mybir.ActivationFunctionType.Square
    Alu = mybir.AluOpType

    with tile.TileContext(nc) as tc, ExitStack() as ctx:
        xp = ctx.enter_context(tc.tile_pool(name="x", bufs=BUFS[0]))
        yp = ctx.enter_context(tc.tile_pool(name="y", bufs=BUFS[1]))
        wp = ctx.enter_context(tc.tile_pool(name="work", bufs=BUFS[2]))
        sp = ctx.enter_context(tc.tile_pool(name="scratch", bufs=BUFS[3]))
        ap_ = ctx.enter_context(tc.tile_pool(name="acc", bufs=1))

        # interleaved acc layout: columns [2g, 2g+1] = (dm-side, xm-side)
        acc = ap_.tile([P, 2 * N_GROUPS], f32)

        tail_ttr = []
        off = 0
        gdmx = gxm = None
        gdoff = gxoff = 0
        for t, cells in enumerate(TILE_SIZES):
            fd = cells * F
            g, k_in_g = _tile_group[t]
            gn = GROUP_OF[g]
            gcells = sum(TILE_SIZES[t - k_in_g:t - k_in_g + gn])
            xt = xp.tile([P, fd], f32, tag="xt")
            yt = yp.tile([P, fd], f32, tag="yt")
            sl = slice(off, off + fd)
            off += fd
            nc.sync.dma_start(yt[:], y_d[:, sl])
            nc.sync.dma_start(xt[:], x_d[:, sl])

            if k_in_g == 0:
                # group buffers: dmx = [dm(t0)|xs0(t0)|dm(t1)|xs0(t1)|...],
                # gxm = [xm(t0)|xm(t1)|...]
                gdmx = wp.tile([P, (gcells * F) + gcells], bf16, tag="dmx")
                gxm = wp.tile([P, gcells * F], bf16, tag="xmg")
                gdoff = gxoff = 0

            # bf16 mask replicated to all 5 features (contiguous); emitted
            # before xs0 because dm (critical path) waits on it
            m5 = wp.tile([P, fd], bf16, tag="m5")
            y0b = yt[:, 0::F].unsqueeze(2).broadcast_to((P, cells, F))
            m5_eng = nc.vector if t in M5_ON_DVE else nc.gpsimd
            m5_eng.tensor_scalar(
                m5[:].rearrange("p (k f) -> p k f", f=F), y0b,
                THRESH, None, op0=Alu.is_gt)

            # GpSimd: xs0 = sqrt(a)*x0 into this tile's dmx tail slice
            nc.gpsimd.tensor_scalar(
                gdmx[:, gdoff + fd:gdoff + fd + cells], xt[:, 0::F],
                sqa, None, op0=Alu.mult)

            # DVE: d = y - x (bf16 out), dm = d*m5 (bf16 2x), xm = x*m5
            dt_ = wp.tile([P, fd], bf16, tag="d")
            nc.vector.tensor_tensor(dt_[:], yt[:], xt[:], op=Alu.subtract)
            nc.vector.tensor_tensor(gdmx[:, gdoff:gdoff + fd], dt_[:], m5[:],
                                    op=Alu.mult)
            xm_eng = nc.gpsimd if t in XM_ON_POOL else nc.vector
            xm_eng.tensor_tensor(gxm[:, gxoff:gxoff + fd], xt[:], m5[:],
                                 op=Alu.mult)
            gdoff += fd + cells
            gxoff += fd

            if k_in_g == gn - 1:
                # group complete: fused square + row-sum over group buffers
                sq = sp.tile([P, gcells * F + gcells], bf16, tag="sq")
                nc.scalar.activation(sq[:], gdmx[:], Sq,
                                     accum_out=acc[:, 2 * g:2 * g + 1])
                if g >= N_GROUPS - TTR_TAIL:
                    tail_ttr.append((g, gxm, gcells))
                elif g in SQ2_ON_DVE:
                    sq2 = sp.tile([P, gcells * F], bf16, tag="sq2")
                    nc.vector.scalar_tensor_tensor(
                        sq2[:], gxm[:], 1.0, gxm[:], op0=Alu.mult,
                        op1=Alu.mult, accum_out=acc[:, 2 * g + 1:2 * g + 2])
                else:
                    sq2 = sp.tile([P, gcells * F], bf16, tag="sq2")
                    nc.scalar.activation(sq2[:], gxm[:], Sq,
                                         accum_out=acc[:, 2 * g + 1:2 * g + 2])

        if REV_TTR:
            tail_ttr = tail_ttr[::-1]
        for (g, gxm, gc) in tail_ttr:
            # xm * 1 * xm summed per row == sum(xm^2); runs on DVE, deferred
            # past the loop so the last tiles' d/dm (which gate ACT) go first
            # (scalar_tensor_tensor is Pool-invalid but DVE-valid on HW)
            sq2 = sp.tile([P, gc * F], bf16, tag="sq2")
            nc.vector.scalar_tensor_tensor(
                sq2[:], gxm[:], 1.0, gxm[:],
                op0=Alu.mult, op1=Alu.mult, accum_out=acc[:, 2 * g + 1:2 * g + 2])

        nc.sync.dma_start(o_d[:], acc[:])

    nc.compile()
    return nc


def _shard(a: np.ndarray) -> list[np.ndarray]:
    flat = a.reshape(-1)
    pad = np.zeros(PAD_CELLS * F, dtype=a.dtype)
    flat = np.concatenate([flat, pad])
    per_core = flat.reshape(N_CORES, P, FD)
    return [np.ascontiguousarray(per_core[i]) for i in range(N_CORES)]


def kernel(x: np.ndarray, y: np.ndarray) -> np.ndarray:
    global _compiled
    if _compiled is None:
        _compiled = _build()
    nc = _compiled

    from concourse.bass_utils import run_bass_kernel_spmd

    xs = _shard(np.asarray(x, dtype=np.float32))
    ys = _shard(np.asarray(y, dtype=np.float32))
    in_maps = [{"x": xs[i], "y": ys[i]} for i in range(N_CORES)]
    res = run_bass_kernel_spmd(nc, in_maps, core_ids=list(range(N_CORES)))

    total = np.float64(0.0)
    for r in res.results:
        o = r["o"].astype(np.float64).reshape(P, 2 * N_GROUPS)
        total += o[:, 0::2].sum()
        total -= ALPHA * o[:, 1::2].sum()
    return np.float32(total)
